# revision 18
# baseline (speedup 1.0000x reference)
"""Trainium2 Bass kernel for nn_ARTLayer (gnn_message_passing).

Math (reference):
    j(i,t) = t + (t>=i)                                    # [K, K-1] neighbor index
    alpha  = sigmoid(x@wa [i] + x@wb [j] + pf@wc + b_att)  # [K, K-1]
    msgs   = mean_t alpha * ((x@WobjT + b_obj)[j] + pf@WpairT + b_pair)
    out    = LN(x + msgs); out = LN(out + FFN(out))

Algebraic rewrite (as v2): U = sum_t a*pf -> U@WpT; Gx via xlo + masked dx;
sum_t a * biases -> s_alpha * bop.

V3 schedule (from NTFF analysis of the 65.5us v2; DVE is the binding
engine at 42.5us busy):
  - sa/sb move to PE (wa/wb col-stationary over xT / xiT blocks); sb_hi is
    a +1 column offset on the sb row (kills shift matmuls + sb stt's).
  - Epilogue arithmetic moves into PSUM accumulation: x residual (id
    matmuls), s*bop / lb2 / b1p rank-1s, cen*ln_g (diag matmul); LN reads
    PSUM directly.
  - Last chunk processed in i-quarters; per-bank U extraction (ACT copy +
    sel matmul) pipelines behind each quarter's sigmoid.
  - gs/dx correction matmuls lag two chunks; stragglers drain into the
    ucp wait gaps.  Junk matmuls keep the PE HAM-warm through the tail;
    dummy sqrt after the last sigmoid hides the ACT table switch.
  - One DMA queue, ordered: b16, batt, pf c0, xT+xlo01, pf c1,
    dx01+xlo23, pf c2, pf c3, dx23, WpT, rows, WoT, dgT, W1T, W2T.
"""
import numpy as np
import ml_dtypes

import concourse.bass as bass
import concourse.tile as tile
from concourse import bacc, mybir

F32, F16 = mybir.dt.float32, mybir.dt.bfloat16
AX = mybir.AxisListType
OP = mybir.AluOpType
AF = mybir.ActivationFunctionType

DEBUG = False
K, D, H, PD = 512, 512, 512, 128
T = K - 1
NCORES, IPC = 8, 64
NCH = 4
EPS = 1e-5

# b16 pack (f16, [128, 793]):
MGE0, ONES0, SEL0, ID0, WAB0, XIT0, POIS0 = 0, 256, 384, 400, 528, 536, 792
B16C = 793


def build_program() -> bacc.Bacc:
    nc = bacc.Bacc("TRN2", target_bir_lowering=False, debug=False)

    def inp(name, shape, dt):
        return nc.dram_tensor(name, shape, dt, kind="ExternalInput").ap()

    pf = inp("pf", [NCH, 128, IPC, PD], F16)     # [chunk, t, i, pd]
    b16 = inp("b16", [128, B16C], F16)
    batt = inp("batt", [1, 1], F32)
    rows = inp("rows", [1, 5, 512], F16)         # [g, bb, lb2, bop/511, b1p]
    grows = inp("grows", [2, 512], F32)          # [ln_g, ln_b] for broadcast
    f32pk = inp("f32pk", [128, 384], F32)        # [xiT32 256 | id32 128]
    xT = inp("xT", [128, 4, K], F16)             # x^T d-blocks [d%128, d//128, t]
    xlo_ch = inp("xlo_ch", [128, NCH, D], F16)
    dxf = inp("dxf", [128, NCH, D], F16)
    dgT = inp("dgT", [128, 4, 128], F16)         # diag(ln_g) blocks
    WpT = inp("WpT", [PD, H], F16)
    WoT = inp("WoT", [D, H], F16)
    W1T = inp("W1T", [H, H], F16)
    W2T = inp("W2T", [H, H], F16)

    out_d = nc.dram_tensor("out", [IPC, H], F32, kind="ExternalOutput").ap()
    if DEBUG:
        dbg_alpha = nc.dram_tensor(
            "dbg_alpha", [128, NCH, IPC], F16, kind="ExternalOutput").ap()
        dbg_h = nc.dram_tensor("dbg_h", [IPC, H], F32, kind="ExternalOutput").ap()
        dbg_u = nc.dram_tensor("dbg_u", [128, IPC], F16, kind="ExternalOutput").ap()
        dbg_gxT = nc.dram_tensor(
            "dbg_gxT", [128, 4, IPC], F16, kind="ExternalOutput").ap()
        dbg_sb = nc.dram_tensor("dbg_sb", [1, 520], F16, kind="ExternalOutput").ap()
        dbg_cen = nc.dram_tensor("dbg_cen", [IPC, H], F16, kind="ExternalOutput").ap()
        dbg_f1 = nc.dram_tensor("dbg_f1", [128, 4, IPC], F16, kind="ExternalOutput").ap()
        dbg_h2 = nc.dram_tensor("dbg_h2", [IPC, H], F32, kind="ExternalOutput").ap()
        dbg_gsb = nc.dram_tensor("dbg_gsb", [IPC, H], F32, kind="ExternalOutput").ap()

    with tile.TileContext(nc) as tc:
        with (
            tc.tile_pool(name="const", bufs=1) as cpool,
            tc.tile_pool(name="pfp", bufs=4) as pfp,
            tc.tile_pool(name="scrp", bufs=2) as scrp,
            tc.tile_pool(name="smallp", bufs=4) as smallp,
            tc.tile_pool(name="postp", bufs=3) as postp,
            tc.tile_pool(name="pss", bufs=2, space="PSUM") as pss,
            tc.tile_pool(name="psu", bufs=4, space="PSUM") as psu,
            tc.tile_pool(name="psg", bufs=1, space="PSUM") as psg,
            tc.tile_pool(name="psm", bufs=1, space="PSUM") as psm,
        ):
            def dma(out, in_):
                nc.sync.dma_start(out=out, in_=in_)

            def dma_w(out, in_):
                nc.scalar.dma_start(out=out, in_=in_)

            def late_dma(dst, src_dram, when):
                # pin the weight DMA behind `when` via a 1-elem gpsimd copy
                nc.gpsimd.tensor_copy(dst[0:1, 0:1], when)
                nc.gpsimd.dma_start(out=dst, in_=src_dram)

            # ---------------- DMA stream (single queue, FIFO) ----------------
            b16_sb = cpool.tile([128, B16C], F16)
            batt_sb = cpool.tile([1, 1], F32)
            dma_w(batt_sb, batt)
            f32pk_sb = cpool.tile([128, 384], F32)
            dma_w(f32pk_sb, f32pk)
            pf_t = []
            for c in range(NCH):
                t_ = pfp.tile([128, IPC, PD], F16, tag="pf_t", name=f"pf{c}")
                pf_t.append(t_)

            def dma_pf(c, pieces):
                w = IPC // pieces
                for k in range(pieces):
                    dma(pf_t[c][:, w * k:w * (k + 1), :],
                        pf[c, :, w * k:w * (k + 1), :])

            dma_pf(0, 4)
            dma(b16_sb, b16)
            xT_sb = cpool.tile([128, 4, K], F16)
            dma(xT_sb, xT)
            xlo = cpool.tile([128, NCH, D], F16)
            dma(xlo[:, 0:2, :], xlo_ch[:, 0:2, :])
            dma_pf(1, 1)
            dx = cpool.tile([128, NCH, D], F16)
            dma(dx[:, 0:2, :], dxf[:, 0:2, :])
            dma(xlo[:, 2:4, :], xlo_ch[:, 2:4, :])
            dma_pf(2, 1)
            dma_pf(3, 4)
            dma(dx[:, 2:4, :], dxf[:, 2:4, :])
            rows_sb = cpool.tile([1, 5, 512], F16)
            dma_w(rows_sb, rows)
            WpT_sb = cpool.tile([PD, H], F16)
            WoT_sb = cpool.tile([128, NCH, H], F16)
            dgT_sb = cpool.tile([128, 4, 128], F16)
            W1T_sb = cpool.tile([128, NCH, H], F16)
            W2T_sb = cpool.tile([128, NCH, H], F16)

            mge_sb = b16_sb[:, MGE0:MGE0 + 256].rearrange(
                "p (c i) -> p c i", c=NCH)
            ones_blk = b16_sb[:, ONES0:ONES0 + 128]
            ones_col = ones_blk[:, 0:1]
            ones_row = ones_blk[0:1, :]
            sel_sb = b16_sb[:, SEL0:SEL0 + 16].rearrange(
                "p (j s) -> p j s", j=4)
            id16_sb = b16_sb[:, ID0:ID0 + 128]
            wab_sb = b16_sb[:, WAB0:WAB0 + 8]
            xiT_sb = b16_sb[:, XIT0:XIT0 + 256].rearrange(
                "p (c i) -> p c i", c=4)
            g_row = rows_sb[:, 0, :]
            bb_row = rows_sb[:, 1, :]
            lb2_row = rows_sb[:, 2, :]
            bop_row = rows_sb[:, 3, :]
            b1p_row = rows_sb[:, 4, :]

            # ---------------- shadow-time setup ----------------
            eps_col = cpool.tile([IPC, 1], F32)
            nc.vector.memset(eps_col, EPS)
            sab_sb = cpool.tile([1, 520], F16)
            nc.vector.memset(sab_sb[:, 512:520], 0.0)
            u_ps = [psu.tile([128, 512], F32, tag="flex", name=f"u_ps{b}")
                    for b in range(4)]
            for b in range(4):
                nc.scalar.memzero(u_ps[b])
            gs_ps = psg.tile([128, 5, IPC], F32)
            nc.scalar.memzero(gs_ps)
            msg_ps = psm.tile([IPC, H], F32)
            nc.vector.memset(msg_ps, 0.0)

            # ---------------- sa/sb via PE ----------------
            sab_ps = pss.tile([1, 512], F32, tag="ps_small", name="sab_ps")
            for db in range(4):
                nc.tensor.matmul(sab_ps, wab_sb[:, 2 * db + 1:2 * db + 2],
                                 xT_sb[:, db, :],
                                 start=(db == 0), stop=(db == 3))
            nc.scalar.copy(sab_sb[:, 0:512], sab_ps)
            sbT_ps = pss.tile([128, 8, 2], F16, tag="ps_small", name="sbT")
            for c in range(NCH):
                nc.tensor.transpose(sbT_ps[:, c, 0:1],
                                    sab_sb[0:1, 128 * c:128 * c + 128],
                                    id16_sb[0:1, 0:1])
                hi = sab_sb[0:1, 128 * c + 1:128 * c + 129]
                nc.tensor.transpose(sbT_ps[:, 4 + c, 0:1], hi,
                                    id16_sb[0:1, 0:1])
            sarow_ps = pss.tile([1, IPC], F32, tag="ps_small", name="sarow")
            for db in range(4):
                nc.tensor.matmul(sarow_ps, wab_sb[:, 2 * db:2 * db + 1],
                                 xiT_sb[:, db, :],
                                 start=(db == 0), stop=(db == 3))

            alpha_full = cpool.tile([128, NCH, IPC], F16)
            age_full = cpool.tile([128, NCH, IPC], F16)

            def tree_full(c, split_l1):
                scr = scrp.tile([128, IPC, 64], F16, tag="scr",
                                name=f"scr{c}")
                if split_l1:
                    for qq in range(4):
                        ih = slice(16 * qq, 16 * qq + 16)
                        nc.vector.tensor_add(
                            scr[:, ih, :], pf_t[c][:, ih, 0:64],
                            pf_t[c][:, ih, 64:128])
                else:
                    nc.vector.tensor_add(scr, pf_t[c][:, :, 0:64],
                                         pf_t[c][:, :, 64:128])
                return scr

            def tree_rest(c, scr, upto=None):
                nc.vector.tensor_add(scr[:, :, 0:32], scr[:, :, 0:32],
                                     scr[:, :, 32:64])
                nc.vector.tensor_add(scr[:, :, 0:16], scr[:, :, 0:16],
                                     scr[:, :, 16:32])
                nc.vector.tensor_add(scr[:, :, 0:8], scr[:, :, 0:8],
                                     scr[:, :, 8:16])
                sc_t = smallp.tile([128, IPC], F32, tag="sc_t")
                nc.vector.tensor_reduce(sc_t, scr[:, :, 0:8], axis=AX.X,
                                        op=OP.add)
                return sc_t

            def sig_chunk(c, sc_t, sbj):
                aarg = smallp.tile([128, IPC], F32, tag="aarg")
                nc.vector.tensor_add(aarg, sc_t, sbj[:, c, :])
                nc.scalar.activation(alpha_full[:, c, :], aarg, AF.Sigmoid)

            def age_chunk(c):
                nc.vector.tensor_mul(age_full[:, c, :], alpha_full[:, c, :],
                                     mge_sb[:, c, :])

            def quads(c, qlist, stop):
                for q in qlist:
                    b, sp = divmod(q, 4)
                    nc.tensor.matmul(
                        u_ps[b][32 * sp:32 * sp + 4, :],
                        alpha_full[:, c, 4 * q:4 * q + 4],
                        pf_t[c][:, 4 * q:4 * q + 4, :],
                        start=False, stop=stop,
                        tile_position=(0, 32 * sp), skip_group_check=True)

            def gs_alpha(c):
                for db in range(4):
                    nc.tensor.matmul(gs_ps[:, db, :],
                                     xlo[:, c, 128 * db:128 * db + 128],
                                     alpha_full[:, c, :],
                                     start=False, stop=False,
                                     skip_group_check=True)
                nc.tensor.matmul(gs_ps[0:1, 4, 0:IPC], ones_col,
                                 alpha_full[:, c, :],
                                 start=False, stop=(c == NCH - 1),
                                 skip_group_check=True)

            def gs_dx(c, stop):
                for db in range(4):
                    nc.tensor.matmul(gs_ps[:, db, :],
                                     dx[:, c, 128 * db:128 * db + 128],
                                     age_full[:, c, :],
                                     start=False, stop=stop,
                                     skip_group_check=True)

            # ---------------- main loop, DVE-issue-ordered ----------------
            # chunk 0 tree
            scr0 = tree_full(0, split_l1=True)
            sc0 = tree_rest(0, scr0)
            # chunk 1: L1 first so the sb-chain stall hides
            scr1 = tree_full(1, split_l1=False)

            # sb chain on DVE (PE work above lands ~17.5)
            sbT = smallp.tile([128, 8], F32, tag="sbT_sb")
            nc.vector.tensor_copy(sbT, sbT_ps[:, :, 0])
            nc.vector.tensor_tensor(sbT[:, 3:4], sbT[:, 3:4],
                                    b16_sb[:, POIS0:POIS0 + 1], OP.add)
            dneg = smallp.tile([128, NCH], F32, tag="dneg")
            nc.vector.tensor_tensor(dneg, sbT[:, 4:8], sbT[:, 0:4],
                                    OP.subtract)
            sa_i = cpool.tile([1, IPC], F16)
            nc.vector.tensor_scalar(sa_i, sarow_ps, batt_sb[0:1, 0:1], None,
                                    OP.add, OP.bypass)
            x_ps = pss.tile([128, IPC], F32, tag="ps_small", name="x_ps")
            nc.tensor.matmul(x_ps, ones_row, sa_i, start=True, stop=True)
            sbj = cpool.tile([128, NCH, IPC], F32)
            for c in range(NCH):
                x3c = smallp.tile([128, IPC], F32, tag="x3c")
                nc.vector.tensor_scalar(x3c, x_ps, sbT[:, c:c + 1], None,
                                        OP.add, OP.bypass)
                nc.vector.scalar_tensor_tensor(
                    sbj[:, c, :], mge_sb[:, c, :], dneg[:, c:c + 1], x3c,
                    OP.mult, OP.add)

            sig_chunk(0, sc0, sbj)
            sc1 = tree_rest(1, scr1)
            age_chunk(0)
            quads(0, range(16), stop=False)

            # chunk 2
            scr2 = tree_full(2, split_l1=False)
            sig_chunk(1, sc1, sbj)
            sc2 = tree_rest(2, scr2)
            age_chunk(1)
            quads(1, range(16), stop=False)
            gs_alpha(0)
            # residual x into msg psum (fp32 path for precision)
            xiT32 = f32pk_sb[:, 0:256].rearrange("p (c i) -> p c i", c=4)
            id32 = f32pk_sb[:, 256:384]
            for db in range(4):
                nc.tensor.matmul(msg_ps[:, 128 * db:128 * db + 128],
                                 xiT32[:, db, :],
                                 id32, start=False, stop=False,
                                 skip_group_check=True)

            # chunk 3 quarter trees interleaved with chunk 2 sigmoid
            scr3 = scrp.tile([128, IPC, 64], F16, tag="scr", name="scr3")
            sc3 = smallp.tile([128, IPC], F32, tag="sc3")

            def tree_q(qq):
                iq = slice(16 * qq, 16 * qq + 16)
                nc.vector.tensor_add(scr3[:, iq, :], pf_t[3][:, iq, 0:64],
                                     pf_t[3][:, iq, 64:128])
                nc.vector.tensor_add(scr3[:, iq, 0:32], scr3[:, iq, 0:32],
                                     scr3[:, iq, 32:64])
                nc.vector.tensor_add(scr3[:, iq, 0:16], scr3[:, iq, 0:16],
                                     scr3[:, iq, 16:32])
                nc.vector.tensor_add(scr3[:, iq, 0:8], scr3[:, iq, 0:8],
                                     scr3[:, iq, 8:16])
                nc.vector.tensor_reduce(sc3[:, iq], scr3[:, iq, 0:8],
                                        axis=AX.X, op=OP.add)

            def sig_q(qq):
                iq = slice(16 * qq, 16 * qq + 16)
                aargq = smallp.tile([128, 16], F32, tag="aargq")
                nc.vector.tensor_add(aargq, sc3[:, iq], sbj[:, 3, iq])
                nc.scalar.activation(alpha_full[:, 3, iq], aargq, AF.Sigmoid)

            slots_ps = pss.tile([128, IPC], F32, tag="ps_small", name="slots")
            slots3 = slots_ps.rearrange("p (g s) -> p g s", g=16)
            ucp = [postp.tile([128, 512], F16, tag="u_cp", name=f"ucp{b}")
                   for b in range(4)]

            def extract_bank(b, eng):
                if eng == "act":
                    nc.scalar.copy(ucp[b], u_ps[b])
                else:
                    nc.vector.tensor_copy(ucp[b], u_ps[b])
                for j in range(4):
                    nc.tensor.matmul(slots3[:, 4 * b + j, :],
                                     ucp[b][:, 128 * j:128 * j + 128],
                                     sel_sb[:, j, :],
                                     start=True, stop=True,
                                     skip_group_check=True)

            tree_q(0)
            sig_chunk(2, sc2, sbj)
            sig_q(0)
            age_chunk(2)
            quads(2, range(16), stop=False)
            gs_alpha(1)
            gs_dx(0, stop=False)

            tree_q(1)
            sig_q(1)
            quads(3, range(0, 4), stop=True)
            extract_bank(0, "act")
            # late weights ride the ACT queue behind the first c3 sigmoid
            late_dma(WpT_sb, WpT, alpha_full[0:1, 3, 0:1])
            late_dma(WoT_sb.rearrange("p c h -> p (c h)"),
                     WoT.rearrange("(c p) h -> p c h", p=128),
                     alpha_full[0:1, 3, 0:1])
            late_dma(dgT_sb.rearrange("p c h -> p (c h)"),
                     dgT.rearrange("p c h -> p (c h)"),
                     alpha_full[0:1, 3, 0:1])
            tree_q(2)
            sig_q(2)
            quads(3, range(4, 8), stop=True)
            gs_alpha(2)
            gs_dx(1, stop=False)
            extract_bank(1, "act")
            late_dma(W1T_sb.rearrange("p c h -> p (c h)"),
                     W1T.rearrange("(c p) h -> p c h", p=128),
                     alpha_full[0:1, 3, 32:33])
            gbb_sb = cpool.tile([IPC, 2, H], F32)
            late_dma(gbb_sb.rearrange("p c h -> p (c h)"),
                     grows[None, :, :].to_broadcast([IPC, 2, H]),
                     alpha_full[0:1, 3, 32:33])
            late_dma(W2T_sb.rearrange("p c h -> p (c h)"),
                     W2T.rearrange("(c p) h -> p c h", p=128),
                     alpha_full[0:1, 3, 32:33])
            tree_q(3)
            sig_q(3)
            junk2 = smallp.tile([1, 1], F32, tag="junk2")
            nc.scalar.activation(junk2, alpha_full[0:1, 3, 48:49], AF.Sqrt)
            age_chunk(3)
            quads(3, range(8, 12), stop=True)
            extract_bank(2, "act")
            quads(3, range(12, 16), stop=True)
            extract_bank(3, "dve")
            gs_alpha(3)
            gs_dx(2, stop=False)
            gs_dx(3, stop=True)

            # s row for the bop rank-1
            s_row16 = smallp.tile([1, IPC], F16, tag="s_row")
            nc.vector.tensor_copy(s_row16, gs_ps[0:1, 4, 0:IPC])
            nc.tensor.matmul(msg_ps, s_row16, bop_row, start=False,
                             stop=False, skip_group_check=True)
            u_sb = postp.tile([128, IPC], F16)
            nc.vector.tensor_copy(
                u_sb.rearrange("p (b s j) -> p b s j", b=4, s=4),
                slots_ps.rearrange("p (b j s) -> p b s j", b=4, j=4))
            nc.tensor.matmul(msg_ps, u_sb, WpT_sb, start=False, stop=False,
                             skip_group_check=True)
            gxT_sb = postp.tile([128, 4, IPC], F16)
            nc.vector.tensor_copy(gxT_sb, gs_ps[:, 0:4, :])
            for db in range(4):
                nc.tensor.matmul(msg_ps, gxT_sb[:, db, :], WoT_sb[:, db, :],
                                 start=False, stop=(db == 3),
                                 skip_group_check=True)
            tc.no_sync_barrier()

            # ---------------- LN1 from PSUM ----------------
            def ln_stats(v_ps):
                stats = smallp.tile([IPC, 6], F32, tag="stats")
                nc.vector.bn_stats(out=stats, in_=v_ps)
                mv = smallp.tile([IPC, 2], F32, tag="mv")
                nc.vector.bn_aggr(out=mv, in_=stats)
                std = smallp.tile([IPC, 1], F32, tag="std")
                nc.scalar.activation(std, mv[:, 1:2], AF.Sqrt, bias=eps_col)
                rstd = smallp.tile([IPC, 1], F32, tag="rstd")
                nc.vector.reciprocal(rstd, std)
                return mv, rstd

            mv1, rstd1 = ln_stats(msg_ps)
            cen = postp.tile([IPC, H], F16)
            o1T_ps = pss.tile([128, 4, IPC], F16, tag="ps_small", name="o1T")
            for hh in range(2):
                blk = slice(256 * hh, 256 * hh + 256)
                nc.vector.tensor_scalar(cen[:, blk], msg_ps[:, blk],
                                        mv1[:, 0:1], rstd1,
                                        OP.subtract, OP.mult)
                for db in (2 * hh, 2 * hh + 1):
                    nc.tensor.transpose(o1T_ps[:, db, :],
                                        cen[:, 128 * db:128 * db + 128],
                                        id16_sb[0:IPC, 0:IPC])
            o1T_sb = postp.tile([128, 4, IPC], F16)
            nc.vector.tensor_copy(o1T_sb, o1T_ps)



            # ---------------- FFN ----------------
            f1T_ps = psu.tile([128, 4, IPC], F32, tag="flex", name="f1T")
            for hb in range(4):
                nc.tensor.matmul(f1T_ps[:, hb, :],
                                 b1p_row[:, 128 * hb:128 * hb + 128],
                                 ones_row[:, 0:IPC],
                                 start=True, stop=False, skip_group_check=True)
            for db in range(4):
                for hb in range(4):
                    nc.tensor.matmul(f1T_ps[:, hb, :],
                                     W1T_sb[:, db, 128 * hb:128 * hb + 128],
                                     o1T_sb[:, db, :],
                                     start=False, stop=(db == 3),
                                     skip_group_check=True)
            f1T_sb = postp.tile([128, 4, IPC], F16)
            nc.vector.tensor_scalar_max(f1T_sb, f1T_ps, 0.0)

            # f2/h2 bank: cen*g (diag) + lb2 (rank-1) + FFN2
            f2_ps = psu.tile([IPC, H], F32, tag="flex", name="f2_ps")
            nc.tensor.matmul(f2_ps, ones_row[:, 0:IPC], lb2_row,
                             start=True, stop=False, skip_group_check=True)
            for hb in range(4):
                nc.tensor.matmul(f2_ps[:, 128 * hb:128 * hb + 128],
                                 o1T_sb[:, hb, :], dgT_sb[:, hb, :],
                                 start=False, stop=False, skip_group_check=True)
            for hb in range(4):
                nc.tensor.matmul(f2_ps, f1T_sb[:, hb, :], W2T_sb[:, hb, :],
                                 start=False, stop=(hb == 3),
                                 skip_group_check=True)

            # ---------------- LN2 from PSUM ----------------
            mv2, rstd2 = ln_stats(f2_ps)
            t2 = postp.tile([IPC, H], F32, tag="t2")
            nc.vector.scalar_tensor_tensor(
                t2, f2_ps, mv2[:, 0:1], gbb_sb[:, 0, :], OP.subtract, OP.mult)
            out2 = postp.tile([IPC, H], F32, tag="out2")
            nc.vector.scalar_tensor_tensor(
                out2, t2, rstd2, gbb_sb[:, 1, :], OP.mult, OP.add)

            nc.sync.dma_start(out=out_d, in_=out2)
            if DEBUG:
                nc.sync.dma_start(out=dbg_alpha, in_=alpha_full)
                nc.sync.dma_start(out=dbg_u, in_=u_sb)
                nc.sync.dma_start(out=dbg_gxT, in_=gxT_sb)
                nc.sync.dma_start(out=dbg_sb, in_=sab_sb)
                dbg_h_sb = postp.tile([IPC, H], F32, tag="dbg_h")
                nc.vector.tensor_copy(dbg_h_sb, msg_ps)
                nc.sync.dma_start(out=dbg_h, in_=dbg_h_sb)
                nc.sync.dma_start(out=dbg_cen, in_=cen)
                nc.sync.dma_start(out=dbg_f1, in_=f1T_sb)
                dbg_h2_sb = postp.tile([IPC, H], F32, tag="dbg_h2")
                nc.vector.tensor_copy(dbg_h2_sb, f2_ps)
                nc.sync.dma_start(out=dbg_h2, in_=dbg_h2_sb)
                nc.sync.dma_start(out=dbg_gsb, in_=g_sb)

    return nc


def prep_in_maps(inputs) -> list[dict]:
    x = np.asarray(inputs["x"], np.float32)
    pf = np.asarray(inputs["pair_feats"], np.float32)
    W_att = np.asarray(inputs["W_att"], np.float32)
    b_att = np.asarray(inputs["b_att"], np.float32)
    W_obj = np.asarray(inputs["W_obj"], np.float32)
    b_obj = np.asarray(inputs["b_obj"], np.float32)
    W_pair = np.asarray(inputs["W_pair"], np.float32)
    b_pair = np.asarray(inputs["b_pair"], np.float32)
    ln_g = np.asarray(inputs["ln_g"], np.float32)
    ln_b = np.asarray(inputs["ln_b"], np.float32)
    W1 = np.asarray(inputs["W1"], np.float32)
    b1 = np.asarray(inputs["b1"], np.float32)
    W2 = np.asarray(inputs["W2"], np.float32)
    b2 = np.asarray(inputs["b2"], np.float32)

    wa, wb, wc = W_att[0, :D], W_att[0, D:2 * D], W_att[0, 2 * D:]
    xpad = np.concatenate([x, np.zeros((1, D), np.float32)], axis=0)

    colscale = np.sign(wc) * np.maximum(np.abs(wc), 6e-5)
    colscale[colscale == 0] = 6e-5
    WpT2 = (W_pair.T / colscale[:, None] / T).astype(ml_dtypes.bfloat16)
    WoT2 = (W_obj.T / T).astype(ml_dtypes.bfloat16)
    dxf_np = np.diff(xpad[:K + 1], axis=0)
    b1p = b1 + ln_b @ W1.T

    b16a = np.zeros((128, B16C), ml_dtypes.bfloat16)
    b16a[:, ONES0:ONES0 + 128] = 1.0
    q = np.arange(128)
    for j in range(4):
        for s in range(4):
            b16a[:, SEL0 + 4 * j + s] = (q == 32 * s + j)
    b16a[:, ID0:ID0 + 128] = np.eye(128, dtype=ml_dtypes.bfloat16)
    b16a[127, POIS0] = -60000.0
    wab = np.zeros((128, 8), np.float32)
    for db in range(4):
        wab[:, 2 * db] = wa[128 * db:128 * db + 128]
        wab[:, 2 * db + 1] = wb[128 * db:128 * db + 128]
    b16a[:, WAB0:WAB0 + 8] = wab.astype(ml_dtypes.bfloat16)

    rows_np = np.stack([ln_g, ln_b, ln_b + b2, (b_obj + b_pair) / T,
                        b1p]).astype(ml_dtypes.bfloat16)[None]
    dgT_np = np.zeros((128, 4, 128), ml_dtypes.bfloat16)
    for hb in range(4):
        dgT_np[:, hb, :] = np.diag(ln_g[128 * hb:128 * hb + 128])

    xT_np = np.ascontiguousarray(
        x.T.reshape(4, 128, K).transpose(1, 0, 2)).astype(ml_dtypes.bfloat16)
    xlo_np = np.ascontiguousarray(
        x.reshape(NCH, 128, D).transpose(1, 0, 2)).astype(ml_dtypes.bfloat16)
    dx_np = np.ascontiguousarray(
        dxf_np.reshape(NCH, 128, D).transpose(1, 0, 2)).astype(ml_dtypes.bfloat16)

    base = dict(
        xT=xT_np,
        xlo_ch=xlo_np,
        dxf=dx_np,
        rows=rows_np,
        dgT=dgT_np,
        batt=b_att.reshape(1, 1).astype(np.float32),
        grows=np.stack([ln_g, ln_b]).astype(np.float32),
        WpT=np.ascontiguousarray(WpT2),
        WoT=np.ascontiguousarray(WoT2),
        W1T=np.ascontiguousarray(W1.T * ln_g[:, None]).astype(ml_dtypes.bfloat16),
        W2T=np.ascontiguousarray(W2.T).astype(ml_dtypes.bfloat16),
    )

    pfr = pf.reshape(K, T, PD)
    tgrid = np.arange(128)[:, None] + 128 * np.arange(NCH)[None, :]

    in_maps = []
    for core in range(NCORES):
        ig = np.arange(core * IPC, (core + 1) * IPC)
        mge = ((tgrid[:, :, None] >= ig[None, None, :])
               & (tgrid[:, :, None] <= T - 1)).astype(ml_dtypes.bfloat16)
        shard = np.zeros((NCH * 128, IPC, PD), ml_dtypes.bfloat16)
        shard[:T] = (pfr[ig] * colscale[None, None, :]).transpose(1, 0, 2)
        pf_shard = np.ascontiguousarray(shard.reshape(NCH, 128, IPC, PD))
        f32pk_np = np.zeros((128, 384), np.float32)
        f32pk_np[:, 0:256] = x[ig].T.reshape(4, 128, IPC).transpose(
            1, 0, 2).reshape(128, 256)
        f32pk_np[:, 256:384] = np.eye(128, dtype=np.float32)
        cb16 = b16a.copy()
        cb16[:, MGE0:MGE0 + 256] = mge.reshape(128, NCH * IPC)
        cb16[:, XIT0:XIT0 + 256] = x[ig].T.reshape(4, 128, IPC).transpose(
            1, 0, 2).reshape(128, 256).astype(ml_dtypes.bfloat16)
        m = dict(base)
        m.update(pf=pf_shard, b16=cb16, f32pk=f32pk_np)
        in_maps.append(m)
    return in_maps


_COMPILED = None


def _get_program() -> bacc.Bacc:
    global _COMPILED
    if _COMPILED is None:
        nc = build_program()
        nc.compile()
        _COMPILED = nc
    return _COMPILED


TRACE = False
LAST_RESULT = None


def _install_axon_ntff_hook():
    import sys
    import types
    try:
        from antenv.axon_hooks import get_axon_ntff_profile_hook  # noqa: F401
        return
    except ImportError:
        pass
    from trn_agent_boot.trn_boot import _ntff_profile_via_ctypes
    hook = _ntff_profile_via_ctypes("/opt/axon/libaxon_pjrt.so")
    m = types.ModuleType("antenv.axon_hooks")
    m.get_axon_ntff_profile_hook = lambda: hook
    sys.modules["antenv.axon_hooks"] = m


def kernel(**inputs) -> np.ndarray:
    import concourse.bass_utils as bu
    from concourse.bass_utils import run_bass_kernel_spmd
    global LAST_RESULT
    if TRACE:
        _install_axon_ntff_hook()
        bu.upload_artifacts = lambda tmpdir: str(tmpdir)
    nc = _get_program()
    in_maps = prep_in_maps(inputs)
    res = run_bass_kernel_spmd(nc, in_maps, list(range(NCORES)), trace=TRACE)
    LAST_RESULT = res
    outs = [res.results[c]["out"] for c in range(NCORES)]
    return np.concatenate(outs, axis=0).astype(np.float32)


# revision 19
# speedup vs baseline: 1.1740x; 1.1740x over previous
"""Trainium2 Bass kernel for nn_ARTLayer (gnn_message_passing).

Math (reference):
    j(i,t) = t + (t>=i)                                    # [K, K-1] neighbor index
    alpha  = sigmoid(x@wa [i] + x@wb [j] + pf@wc + b_att)  # [K, K-1]
    msgs   = mean_t alpha * ((x@WobjT + b_obj)[j] + pf@WpairT + b_pair)
    out    = LN(x + msgs); out = LN(out + FFN(out))

Algebraic rewrite (as v2): U = sum_t a*pf -> U@WpT; Gx via xlo + masked dx;
sum_t a * biases -> s_alpha * bop.

V3 schedule (from NTFF analysis of the 65.5us v2; DVE is the binding
engine at 42.5us busy):
  - sa/sb move to PE (wa/wb col-stationary over xT / xiT blocks); sb_hi is
    a +1 column offset on the sb row (kills shift matmuls + sb stt's).
  - Epilogue arithmetic moves into PSUM accumulation: x residual (id
    matmuls), s*bop / lb2 / b1p rank-1s, cen*ln_g (diag matmul); LN reads
    PSUM directly.
  - Last chunk processed in i-quarters; per-bank U extraction (ACT copy +
    sel matmul) pipelines behind each quarter's sigmoid.
  - gs/dx correction matmuls lag two chunks; stragglers drain into the
    ucp wait gaps.  Junk matmuls keep the PE HAM-warm through the tail;
    dummy sqrt after the last sigmoid hides the ACT table switch.
  - One DMA queue, ordered: b16, batt, pf c0, xT+xlo01, pf c1,
    dx01+xlo23, pf c2, pf c3, dx23, WpT, rows, WoT, dgT, W1T, W2T.
"""
import numpy as np
import ml_dtypes

import concourse.bass as bass
import concourse.tile as tile
from concourse import bacc, mybir

F32, F16 = mybir.dt.float32, mybir.dt.bfloat16
AX = mybir.AxisListType
OP = mybir.AluOpType
AF = mybir.ActivationFunctionType

DEBUG = False
K, D, H, PD = 512, 512, 512, 128
T = K - 1
NCORES, IPC = 8, 64
NCH = 4
EPS = 1e-5

# b16 pack (bf16):
MGE0, ONES0, SEL0, ID0, WAB0, XIT0, POIS0, XITC0 = 0, 256, 384, 400, 528, 536, 792, 793
B16C = 1049


def build_program() -> bacc.Bacc:
    nc = bacc.Bacc("TRN2", target_bir_lowering=False, debug=False)

    def inp(name, shape, dt):
        return nc.dram_tensor(name, shape, dt, kind="ExternalInput").ap()

    pf = inp("pf", [NCH, 128, IPC, PD], F16)     # [chunk, t, i, pd]
    b16 = inp("b16", [128, B16C], F16)
    batt = inp("batt", [1, 1], F32)
    rows = inp("rows", [1, 5, 512], F16)         # [g, bb, lb2, bop/511, b1p]
    grows = inp("grows", [2, 512], F32)          # [ln_g, ln_b] for broadcast
    xT = inp("xT", [128, 4, K], F16)             # x^T d-blocks [d%128, d//128, t]
    xlo_ch = inp("xlo_ch", [128, NCH, D], F16)
    dxf = inp("dxf", [128, NCH, D], F16)
    dgT = inp("dgT", [128, 4, 128], F16)         # diag(ln_g) blocks
    WpT = inp("WpT", [PD, H], F16)
    WoT = inp("WoT", [D, H], F16)
    W1T = inp("W1T", [H, H], F16)
    W2T = inp("W2T", [H, H], F16)

    out_d = nc.dram_tensor("out", [IPC, H], F32, kind="ExternalOutput").ap()
    if DEBUG:
        dbg_alpha = nc.dram_tensor(
            "dbg_alpha", [128, NCH, IPC], F16, kind="ExternalOutput").ap()
        dbg_h = nc.dram_tensor("dbg_h", [IPC, H], F32, kind="ExternalOutput").ap()
        dbg_u = nc.dram_tensor("dbg_u", [128, IPC], F16, kind="ExternalOutput").ap()
        dbg_gxT = nc.dram_tensor(
            "dbg_gxT", [128, 4, IPC], F16, kind="ExternalOutput").ap()
        dbg_sb = nc.dram_tensor("dbg_sb", [1, 520], F16, kind="ExternalOutput").ap()
        dbg_cen = nc.dram_tensor("dbg_cen", [IPC, H], F16, kind="ExternalOutput").ap()
        dbg_f1 = nc.dram_tensor("dbg_f1", [128, 4, IPC], F16, kind="ExternalOutput").ap()
        dbg_h2 = nc.dram_tensor("dbg_h2", [IPC, H], F32, kind="ExternalOutput").ap()
        dbg_gsb = nc.dram_tensor("dbg_gsb", [IPC, H], F32, kind="ExternalOutput").ap()

    with tile.TileContext(nc) as tc:
        with (
            tc.tile_pool(name="const", bufs=1) as cpool,
            tc.tile_pool(name="pfp", bufs=4) as pfp,
            tc.tile_pool(name="scrp", bufs=2) as scrp,
            tc.tile_pool(name="smallp", bufs=4) as smallp,
            tc.tile_pool(name="postp", bufs=3) as postp,
            tc.tile_pool(name="pss", bufs=2, space="PSUM") as pss,
            tc.tile_pool(name="psu", bufs=4, space="PSUM") as psu,
            tc.tile_pool(name="psg", bufs=1, space="PSUM") as psg,
            tc.tile_pool(name="psm", bufs=1, space="PSUM") as psm,
        ):
            def dma(out, in_):
                nc.sync.dma_start(out=out, in_=in_)

            def dma_w(out, in_):
                nc.scalar.dma_start(out=out, in_=in_)

            def late_dma(dst, src_dram, when):
                # pin the weight DMA behind `when` via a 1-elem gpsimd copy
                nc.gpsimd.tensor_copy(dst[0:1, 0:1], when)
                nc.gpsimd.dma_start(out=dst, in_=src_dram)

            # ---------------- DMA stream (single queue, FIFO) ----------------
            b16_sb = cpool.tile([128, B16C], F16)
            batt_sb = cpool.tile([1, 1], F32)
            dma_w(batt_sb, batt)
            pf_t = []
            for c in range(NCH):
                t_ = pfp.tile([128, IPC, PD], F16, tag="pf_t", name=f"pf{c}")
                pf_t.append(t_)

            def dma_pf(c, pieces):
                w = IPC // pieces
                for k in range(pieces):
                    dma(pf_t[c][:, w * k:w * (k + 1), :],
                        pf[c, :, w * k:w * (k + 1), :])

            dma_pf(0, 4)
            dma(b16_sb, b16)
            xT_sb = cpool.tile([128, 4, K], F16)
            dma(xT_sb, xT)
            xlo = cpool.tile([128, NCH, D], F16)
            dma(xlo[:, 0:2, :], xlo_ch[:, 0:2, :])
            dma_pf(1, 1)
            dx = cpool.tile([128, NCH, D], F16)
            dma(dx[:, 0:2, :], dxf[:, 0:2, :])
            dma(xlo[:, 2:4, :], xlo_ch[:, 2:4, :])
            dma_pf(2, 1)
            dma_pf(3, 4)
            dma(dx[:, 2:4, :], dxf[:, 2:4, :])
            rows_sb = cpool.tile([1, 5, 512], F16)
            dma_w(rows_sb, rows)
            WpT_sb = cpool.tile([PD, H], F16)
            WoT_sb = cpool.tile([128, NCH, H], F16)
            dgT_sb = cpool.tile([128, 4, 128], F16)
            W1T_sb = cpool.tile([128, NCH, H], F16)
            W2T_sb = cpool.tile([128, NCH, H], F16)

            mge_sb = b16_sb[:, MGE0:MGE0 + 256].rearrange(
                "p (c i) -> p c i", c=NCH)
            ones_blk = b16_sb[:, ONES0:ONES0 + 128]
            ones_col = ones_blk[:, 0:1]
            ones_row = ones_blk[0:1, :]
            sel_sb = b16_sb[:, SEL0:SEL0 + 16].rearrange(
                "p (j s) -> p j s", j=4)
            id16_sb = b16_sb[:, ID0:ID0 + 128]
            wab_sb = b16_sb[:, WAB0:WAB0 + 8]
            xiT_sb = b16_sb[:, XIT0:XIT0 + 256].rearrange(
                "p (c i) -> p c i", c=4)
            xiTc_sb = b16_sb[:, XITC0:XITC0 + 256].rearrange(
                "p (c i) -> p c i", c=4)
            g_row = rows_sb[:, 0, :]
            bb_row = rows_sb[:, 1, :]
            lb2_row = rows_sb[:, 2, :]
            bop_row = rows_sb[:, 3, :]
            b1p_row = rows_sb[:, 4, :]

            # ---------------- shadow-time setup ----------------
            eps_col = cpool.tile([IPC, 1], F32)
            nc.vector.memset(eps_col, EPS)
            sab_sb = cpool.tile([1, 520], F16)
            nc.vector.memset(sab_sb[:, 512:520], 0.0)
            u_ps = [psu.tile([128, 512], F32, tag="flex", name=f"u_ps{b}")
                    for b in range(4)]
            for b in range(4):
                nc.scalar.memzero(u_ps[b])
            gs_ps = psg.tile([128, 5, IPC], F32)
            nc.scalar.memzero(gs_ps)
            msg_ps = psm.tile([IPC, H], F32)
            nc.vector.memset(msg_ps, 0.0)

            # ---------------- sa/sb via PE ----------------
            sab_ps = pss.tile([1, 512], F32, tag="ps_small", name="sab_ps")
            for db in range(4):
                nc.tensor.matmul(sab_ps, wab_sb[:, 2 * db + 1:2 * db + 2],
                                 xT_sb[:, db, :],
                                 start=(db == 0), stop=(db == 3))
            nc.scalar.copy(sab_sb[:, 0:512], sab_ps)
            sbT_ps = pss.tile([128, 8, 2], F16, tag="ps_small", name="sbT")
            for c in range(NCH):
                nc.tensor.transpose(sbT_ps[:, c, 0:1],
                                    sab_sb[0:1, 128 * c:128 * c + 128],
                                    id16_sb[0:1, 0:1])
                hi = sab_sb[0:1, 128 * c + 1:128 * c + 129]
                nc.tensor.transpose(sbT_ps[:, 4 + c, 0:1], hi,
                                    id16_sb[0:1, 0:1])
            sarow_ps = pss.tile([1, IPC], F32, tag="ps_small", name="sarow")
            for db in range(4):
                nc.tensor.matmul(sarow_ps, wab_sb[:, 2 * db:2 * db + 1],
                                 xiT_sb[:, db, :],
                                 start=(db == 0), stop=(db == 3))

            alpha_full = cpool.tile([128, NCH, IPC], F16)
            age_full = cpool.tile([128, NCH, IPC], F16)

            def tree_full(c, split_l1):
                scr = scrp.tile([128, IPC, 64], F16, tag="scr",
                                name=f"scr{c}")
                if split_l1:
                    for qq in range(4):
                        ih = slice(16 * qq, 16 * qq + 16)
                        nc.vector.tensor_add(
                            scr[:, ih, :], pf_t[c][:, ih, 0:64],
                            pf_t[c][:, ih, 64:128])
                else:
                    nc.vector.tensor_add(scr, pf_t[c][:, :, 0:64],
                                         pf_t[c][:, :, 64:128])
                return scr

            def tree_rest(c, scr, upto=None):
                nc.vector.tensor_add(scr[:, :, 0:32], scr[:, :, 0:32],
                                     scr[:, :, 32:64])
                nc.vector.tensor_add(scr[:, :, 0:16], scr[:, :, 0:16],
                                     scr[:, :, 16:32])
                nc.vector.tensor_add(scr[:, :, 0:8], scr[:, :, 0:8],
                                     scr[:, :, 8:16])
                sc_t = smallp.tile([128, IPC], F32, tag="sc_t")
                nc.vector.tensor_reduce(sc_t, scr[:, :, 0:8], axis=AX.X,
                                        op=OP.add)
                return sc_t

            def sig_chunk(c, sc_t, sbj):
                aarg = smallp.tile([128, IPC], F32, tag="aarg")
                nc.vector.tensor_add(aarg, sc_t, sbj[:, c, :])
                nc.scalar.activation(alpha_full[:, c, :], aarg, AF.Sigmoid)

            def age_chunk(c):
                nc.vector.tensor_mul(age_full[:, c, :], alpha_full[:, c, :],
                                     mge_sb[:, c, :])

            def quads(c, qlist, stop):
                for q in qlist:
                    b, sp = divmod(q, 4)
                    nc.tensor.matmul(
                        u_ps[b][32 * sp:32 * sp + 4, :],
                        alpha_full[:, c, 4 * q:4 * q + 4],
                        pf_t[c][:, 4 * q:4 * q + 4, :],
                        start=False, stop=stop,
                        tile_position=(0, 32 * sp), skip_group_check=True)

            def gs_alpha(c):
                for db in range(4):
                    nc.tensor.matmul(gs_ps[:, db, :],
                                     xlo[:, c, 128 * db:128 * db + 128],
                                     alpha_full[:, c, :],
                                     start=False, stop=False,
                                     skip_group_check=True)
                nc.tensor.matmul(gs_ps[0:1, 4, 0:IPC], ones_col,
                                 alpha_full[:, c, :],
                                 start=False, stop=(c == NCH - 1),
                                 skip_group_check=True)

            def gs_dx(c, stop):
                for db in range(4):
                    nc.tensor.matmul(gs_ps[:, db, :],
                                     dx[:, c, 128 * db:128 * db + 128],
                                     age_full[:, c, :],
                                     start=False, stop=stop,
                                     skip_group_check=True)

            # ---------------- main loop, DVE-issue-ordered ----------------
            # chunk 0 tree
            scr0 = tree_full(0, split_l1=True)
            sc0 = tree_rest(0, scr0)
            # chunk 1: L1 first so the sb-chain stall hides
            scr1 = tree_full(1, split_l1=False)

            # sb chain on DVE (PE work above lands ~17.5)
            sbT = smallp.tile([128, 8], F32, tag="sbT_sb")
            nc.vector.tensor_copy(sbT, sbT_ps[:, :, 0])
            nc.vector.tensor_tensor(sbT[:, 3:4], sbT[:, 3:4],
                                    b16_sb[:, POIS0:POIS0 + 1], OP.add)
            dneg = smallp.tile([128, NCH], F32, tag="dneg")
            nc.vector.tensor_tensor(dneg, sbT[:, 4:8], sbT[:, 0:4],
                                    OP.subtract)
            sa_i = cpool.tile([1, IPC], F16)
            nc.vector.tensor_scalar(sa_i, sarow_ps, batt_sb[0:1, 0:1], None,
                                    OP.add, OP.bypass)
            x_ps = pss.tile([128, IPC], F32, tag="ps_small", name="x_ps")
            nc.tensor.matmul(x_ps, ones_row, sa_i, start=True, stop=True)
            sbj = cpool.tile([128, NCH, IPC], F32)
            for c in range(NCH):
                x3c = smallp.tile([128, IPC], F32, tag="x3c")
                nc.vector.tensor_scalar(x3c, x_ps, sbT[:, c:c + 1], None,
                                        OP.add, OP.bypass)
                nc.vector.scalar_tensor_tensor(
                    sbj[:, c, :], mge_sb[:, c, :], dneg[:, c:c + 1], x3c,
                    OP.mult, OP.add)

            sig_chunk(0, sc0, sbj)
            sc1 = tree_rest(1, scr1)
            age_chunk(0)
            quads(0, range(16), stop=False)

            # chunk 2
            scr2 = tree_full(2, split_l1=False)
            sig_chunk(1, sc1, sbj)
            sc2 = tree_rest(2, scr2)
            age_chunk(1)
            quads(1, range(16), stop=False)
            gs_alpha(0)
            # residual x into msg psum (double-bf16: value + residual)
            for db in range(4):
                nc.tensor.matmul(msg_ps[:, 128 * db:128 * db + 128],
                                 xiT_sb[:, db, :],
                                 id16_sb, start=False, stop=False,
                                 skip_group_check=True)
                nc.tensor.matmul(msg_ps[:, 128 * db:128 * db + 128],
                                 xiTc_sb[:, db, :],
                                 id16_sb, start=False, stop=False,
                                 skip_group_check=True)

            # chunk 3 quarter trees interleaved with chunk 2 sigmoid
            scr3 = scrp.tile([128, IPC, 64], F16, tag="scr", name="scr3")
            sc3 = smallp.tile([128, IPC], F32, tag="sc3")

            def tree_q(qq):
                iq = slice(16 * qq, 16 * qq + 16)
                nc.vector.tensor_add(scr3[:, iq, :], pf_t[3][:, iq, 0:64],
                                     pf_t[3][:, iq, 64:128])
                nc.vector.tensor_add(scr3[:, iq, 0:32], scr3[:, iq, 0:32],
                                     scr3[:, iq, 32:64])
                nc.vector.tensor_add(scr3[:, iq, 0:16], scr3[:, iq, 0:16],
                                     scr3[:, iq, 16:32])
                nc.vector.tensor_add(scr3[:, iq, 0:8], scr3[:, iq, 0:8],
                                     scr3[:, iq, 8:16])
                nc.vector.tensor_reduce(sc3[:, iq], scr3[:, iq, 0:8],
                                        axis=AX.X, op=OP.add)

            def sig_q(qq):
                iq = slice(16 * qq, 16 * qq + 16)
                aargq = smallp.tile([128, 16], F32, tag="aargq")
                nc.vector.tensor_add(aargq, sc3[:, iq], sbj[:, 3, iq])
                nc.scalar.activation(alpha_full[:, 3, iq], aargq, AF.Sigmoid)

            slots_ps = pss.tile([128, IPC], F32, tag="ps_small", name="slots")
            slots3 = slots_ps.rearrange("p (g s) -> p g s", g=16)
            ucp = [postp.tile([128, 512], F16, tag="u_cp", name=f"ucp{b}")
                   for b in range(4)]

            def extract_bank(b, eng):
                if eng == "act":
                    nc.scalar.copy(ucp[b], u_ps[b])
                else:
                    nc.vector.tensor_copy(ucp[b], u_ps[b])
                for j in range(4):
                    nc.tensor.matmul(slots3[:, 4 * b + j, :],
                                     ucp[b][:, 128 * j:128 * j + 128],
                                     sel_sb[:, j, :],
                                     start=True, stop=True,
                                     skip_group_check=True)

            tree_q(0)
            sig_chunk(2, sc2, sbj)
            sig_q(0)
            age_chunk(2)
            quads(2, range(16), stop=False)
            gs_alpha(1)
            gs_dx(0, stop=False)

            tree_q(1)
            sig_q(1)
            quads(3, range(0, 4), stop=True)
            extract_bank(0, "act")
            # late weights ride the ACT queue behind the first c3 sigmoid
            late_dma(WpT_sb, WpT, alpha_full[0:1, 3, 0:1])
            late_dma(WoT_sb.rearrange("p c h -> p (c h)"),
                     WoT.rearrange("(c p) h -> p c h", p=128),
                     alpha_full[0:1, 3, 0:1])
            late_dma(dgT_sb.rearrange("p c h -> p (c h)"),
                     dgT.rearrange("p c h -> p (c h)"),
                     alpha_full[0:1, 3, 0:1])
            tree_q(2)
            sig_q(2)
            quads(3, range(4, 8), stop=True)
            gs_alpha(2)
            gs_dx(1, stop=False)
            extract_bank(1, "act")
            late_dma(W1T_sb.rearrange("p c h -> p (c h)"),
                     W1T.rearrange("(c p) h -> p c h", p=128),
                     alpha_full[0:1, 3, 32:33])
            gbb_sb = cpool.tile([IPC, 2, H], F32)
            late_dma(gbb_sb.rearrange("p c h -> p (c h)"),
                     grows[None, :, :].to_broadcast([IPC, 2, H]),
                     alpha_full[0:1, 3, 32:33])
            late_dma(W2T_sb.rearrange("p c h -> p (c h)"),
                     W2T.rearrange("(c p) h -> p c h", p=128),
                     alpha_full[0:1, 3, 32:33])
            tree_q(3)
            sig_q(3)
            junk2 = smallp.tile([1, 1], F32, tag="junk2")
            nc.scalar.activation(junk2, alpha_full[0:1, 3, 48:49], AF.Sqrt)
            age_chunk(3)
            quads(3, range(8, 12), stop=True)
            extract_bank(2, "act")
            quads(3, range(12, 16), stop=True)
            extract_bank(3, "dve")
            gs_alpha(3)
            gs_dx(2, stop=False)
            gs_dx(3, stop=True)

            # s row for the bop rank-1
            s_row16 = smallp.tile([1, IPC], F16, tag="s_row")
            nc.vector.tensor_copy(s_row16, gs_ps[0:1, 4, 0:IPC])
            nc.tensor.matmul(msg_ps, s_row16, bop_row, start=False,
                             stop=False, skip_group_check=True)
            u_sb = postp.tile([128, IPC], F16)
            nc.vector.tensor_copy(
                u_sb.rearrange("p (b s j) -> p b s j", b=4, s=4),
                slots_ps.rearrange("p (b j s) -> p b s j", b=4, j=4))
            nc.tensor.matmul(msg_ps, u_sb, WpT_sb, start=False, stop=False,
                             skip_group_check=True)
            gxT_sb = postp.tile([128, 4, IPC], F16)
            nc.vector.tensor_copy(gxT_sb, gs_ps[:, 0:4, :])
            for db in range(4):
                nc.tensor.matmul(msg_ps, gxT_sb[:, db, :], WoT_sb[:, db, :],
                                 start=False, stop=(db == 3),
                                 skip_group_check=True)
            tc.no_sync_barrier()

            # ---------------- LN1 from PSUM ----------------
            def ln_stats(v_ps):
                stats = smallp.tile([IPC, 6], F32, tag="stats")
                nc.vector.bn_stats(out=stats, in_=v_ps)
                mv = smallp.tile([IPC, 2], F32, tag="mv")
                nc.vector.bn_aggr(out=mv, in_=stats)
                std = smallp.tile([IPC, 1], F32, tag="std")
                nc.scalar.activation(std, mv[:, 1:2], AF.Sqrt, bias=eps_col)
                rstd = smallp.tile([IPC, 1], F32, tag="rstd")
                nc.vector.reciprocal(rstd, std)
                return mv, rstd

            mv1, rstd1 = ln_stats(msg_ps)
            cen = postp.tile([IPC, H], F16)
            o1T_ps = pss.tile([128, 4, IPC], F16, tag="ps_small", name="o1T")
            for hh in range(2):
                blk = slice(256 * hh, 256 * hh + 256)
                nc.vector.tensor_scalar(cen[:, blk], msg_ps[:, blk],
                                        mv1[:, 0:1], rstd1,
                                        OP.subtract, OP.mult)
                for db in (2 * hh, 2 * hh + 1):
                    nc.tensor.transpose(o1T_ps[:, db, :],
                                        cen[:, 128 * db:128 * db + 128],
                                        id16_sb[0:IPC, 0:IPC])
            o1T_sb = postp.tile([128, 4, IPC], F16)
            nc.vector.tensor_copy(o1T_sb, o1T_ps)



            # ---------------- FFN ----------------
            f1T_ps = psu.tile([128, 4, IPC], F32, tag="flex", name="f1T")
            for hb in range(4):
                nc.tensor.matmul(f1T_ps[:, hb, :],
                                 b1p_row[:, 128 * hb:128 * hb + 128],
                                 ones_row[:, 0:IPC],
                                 start=True, stop=False, skip_group_check=True)
            for db in range(4):
                for hb in range(4):
                    nc.tensor.matmul(f1T_ps[:, hb, :],
                                     W1T_sb[:, db, 128 * hb:128 * hb + 128],
                                     o1T_sb[:, db, :],
                                     start=False, stop=(db == 3),
                                     skip_group_check=True)
            f1T_sb = postp.tile([128, 4, IPC], F16)
            nc.vector.tensor_scalar_max(f1T_sb, f1T_ps, 0.0)

            # f2/h2 bank: cen*g (diag) + lb2 (rank-1) + FFN2
            f2_ps = psu.tile([IPC, H], F32, tag="flex", name="f2_ps")
            nc.tensor.matmul(f2_ps, ones_row[:, 0:IPC], lb2_row,
                             start=True, stop=False, skip_group_check=True)
            for hb in range(4):
                nc.tensor.matmul(f2_ps[:, 128 * hb:128 * hb + 128],
                                 o1T_sb[:, hb, :], dgT_sb[:, hb, :],
                                 start=False, stop=False, skip_group_check=True)
            for hb in range(4):
                nc.tensor.matmul(f2_ps, f1T_sb[:, hb, :], W2T_sb[:, hb, :],
                                 start=False, stop=(hb == 3),
                                 skip_group_check=True)

            # ---------------- LN2 from PSUM ----------------
            mv2, rstd2 = ln_stats(f2_ps)
            t2 = postp.tile([IPC, H], F32, tag="t2")
            nc.vector.scalar_tensor_tensor(
                t2, f2_ps, mv2[:, 0:1], gbb_sb[:, 0, :], OP.subtract, OP.mult)
            out2 = postp.tile([IPC, H], F32, tag="out2")
            nc.vector.scalar_tensor_tensor(
                out2, t2, rstd2, gbb_sb[:, 1, :], OP.mult, OP.add)

            nc.sync.dma_start(out=out_d, in_=out2)
            if DEBUG:
                nc.sync.dma_start(out=dbg_alpha, in_=alpha_full)
                nc.sync.dma_start(out=dbg_u, in_=u_sb)
                nc.sync.dma_start(out=dbg_gxT, in_=gxT_sb)
                nc.sync.dma_start(out=dbg_sb, in_=sab_sb)
                dbg_h_sb = postp.tile([IPC, H], F32, tag="dbg_h")
                nc.vector.tensor_copy(dbg_h_sb, msg_ps)
                nc.sync.dma_start(out=dbg_h, in_=dbg_h_sb)
                nc.sync.dma_start(out=dbg_cen, in_=cen)
                nc.sync.dma_start(out=dbg_f1, in_=f1T_sb)
                dbg_h2_sb = postp.tile([IPC, H], F32, tag="dbg_h2")
                nc.vector.tensor_copy(dbg_h2_sb, f2_ps)
                nc.sync.dma_start(out=dbg_h2, in_=dbg_h2_sb)
                nc.sync.dma_start(out=dbg_gsb, in_=g_sb)

    return nc


def prep_in_maps(inputs) -> list[dict]:
    x = np.asarray(inputs["x"], np.float32)
    pf = np.asarray(inputs["pair_feats"], np.float32)
    W_att = np.asarray(inputs["W_att"], np.float32)
    b_att = np.asarray(inputs["b_att"], np.float32)
    W_obj = np.asarray(inputs["W_obj"], np.float32)
    b_obj = np.asarray(inputs["b_obj"], np.float32)
    W_pair = np.asarray(inputs["W_pair"], np.float32)
    b_pair = np.asarray(inputs["b_pair"], np.float32)
    ln_g = np.asarray(inputs["ln_g"], np.float32)
    ln_b = np.asarray(inputs["ln_b"], np.float32)
    W1 = np.asarray(inputs["W1"], np.float32)
    b1 = np.asarray(inputs["b1"], np.float32)
    W2 = np.asarray(inputs["W2"], np.float32)
    b2 = np.asarray(inputs["b2"], np.float32)

    wa, wb, wc = W_att[0, :D], W_att[0, D:2 * D], W_att[0, 2 * D:]
    xpad = np.concatenate([x, np.zeros((1, D), np.float32)], axis=0)

    colscale = np.sign(wc) * np.maximum(np.abs(wc), 6e-5)
    colscale[colscale == 0] = 6e-5
    WpT2 = (W_pair.T / colscale[:, None] / T).astype(ml_dtypes.bfloat16)
    WoT2 = (W_obj.T / T).astype(ml_dtypes.bfloat16)
    dxf_np = np.diff(xpad[:K + 1], axis=0)
    b1p = b1 + ln_b @ W1.T

    b16a = np.zeros((128, B16C), ml_dtypes.bfloat16)
    b16a[:, ONES0:ONES0 + 128] = 1.0
    q = np.arange(128)
    for j in range(4):
        for s in range(4):
            b16a[:, SEL0 + 4 * j + s] = (q == 32 * s + j)
    b16a[:, ID0:ID0 + 128] = np.eye(128, dtype=ml_dtypes.bfloat16)
    b16a[127, POIS0] = -60000.0
    wab = np.zeros((128, 8), np.float32)
    for db in range(4):
        wab[:, 2 * db] = wa[128 * db:128 * db + 128]
        wab[:, 2 * db + 1] = wb[128 * db:128 * db + 128]
    b16a[:, WAB0:WAB0 + 8] = wab.astype(ml_dtypes.bfloat16)

    rows_np = np.stack([ln_g, ln_b, ln_b + b2, (b_obj + b_pair) / T,
                        b1p]).astype(ml_dtypes.bfloat16)[None]
    dgT_np = np.zeros((128, 4, 128), ml_dtypes.bfloat16)
    for hb in range(4):
        dgT_np[:, hb, :] = np.diag(ln_g[128 * hb:128 * hb + 128])

    xT_np = np.ascontiguousarray(
        x.T.reshape(4, 128, K).transpose(1, 0, 2)).astype(ml_dtypes.bfloat16)
    xlo_np = np.ascontiguousarray(
        x.reshape(NCH, 128, D).transpose(1, 0, 2)).astype(ml_dtypes.bfloat16)
    dx_np = np.ascontiguousarray(
        dxf_np.reshape(NCH, 128, D).transpose(1, 0, 2)).astype(ml_dtypes.bfloat16)

    base = dict(
        xT=xT_np,
        xlo_ch=xlo_np,
        dxf=dx_np,
        rows=rows_np,
        dgT=dgT_np,
        batt=b_att.reshape(1, 1).astype(np.float32),
        grows=np.stack([ln_g, ln_b]).astype(np.float32),
        WpT=np.ascontiguousarray(WpT2),
        WoT=np.ascontiguousarray(WoT2),
        W1T=np.ascontiguousarray(W1.T * ln_g[:, None]).astype(ml_dtypes.bfloat16),
        W2T=np.ascontiguousarray(W2.T).astype(ml_dtypes.bfloat16),
    )

    pfr = pf.reshape(K, T, PD)
    tgrid = np.arange(128)[:, None] + 128 * np.arange(NCH)[None, :]

    in_maps = []
    for core in range(NCORES):
        ig = np.arange(core * IPC, (core + 1) * IPC)
        mge = ((tgrid[:, :, None] >= ig[None, None, :])
               & (tgrid[:, :, None] <= T - 1)).astype(ml_dtypes.bfloat16)
        shard = np.zeros((NCH * 128, IPC, PD), ml_dtypes.bfloat16)
        shard[:T] = (pfr[ig] * colscale[None, None, :]).transpose(1, 0, 2)
        pf_shard = np.ascontiguousarray(shard.reshape(NCH, 128, IPC, PD))
        cb16 = b16a.copy()
        cb16[:, MGE0:MGE0 + 256] = mge.reshape(128, NCH * IPC)
        xiT_f = x[ig].T.reshape(4, 128, IPC).transpose(
            1, 0, 2).reshape(128, 256)
        xiT_b = xiT_f.astype(ml_dtypes.bfloat16)
        cb16[:, XIT0:XIT0 + 256] = xiT_b
        cb16[:, XITC0:XITC0 + 256] = (
            xiT_f - xiT_b.astype(np.float32)).astype(ml_dtypes.bfloat16)
        m = dict(base)
        m.update(pf=pf_shard, b16=cb16)
        in_maps.append(m)
    return in_maps


_COMPILED = None


def _get_program() -> bacc.Bacc:
    global _COMPILED
    if _COMPILED is None:
        nc = build_program()
        nc.compile()
        _COMPILED = nc
    return _COMPILED


TRACE = False
LAST_RESULT = None


def _install_axon_ntff_hook():
    import sys
    import types
    try:
        from antenv.axon_hooks import get_axon_ntff_profile_hook  # noqa: F401
        return
    except ImportError:
        pass
    from trn_agent_boot.trn_boot import _ntff_profile_via_ctypes
    hook = _ntff_profile_via_ctypes("/opt/axon/libaxon_pjrt.so")
    m = types.ModuleType("antenv.axon_hooks")
    m.get_axon_ntff_profile_hook = lambda: hook
    sys.modules["antenv.axon_hooks"] = m


def kernel(**inputs) -> np.ndarray:
    import concourse.bass_utils as bu
    from concourse.bass_utils import run_bass_kernel_spmd
    global LAST_RESULT
    if TRACE:
        _install_axon_ntff_hook()
        bu.upload_artifacts = lambda tmpdir: str(tmpdir)
    nc = _get_program()
    in_maps = prep_in_maps(inputs)
    res = run_bass_kernel_spmd(nc, in_maps, list(range(NCORES)), trace=TRACE)
    LAST_RESULT = res
    outs = [res.results[c]["out"] for c in range(NCORES)]
    return np.concatenate(outs, axis=0).astype(np.float32)
